# revision 16
# baseline (speedup 1.0000x reference)
"""Trainium2 Bass kernel for nn_EvaluatorNetwork.

Network (per sample):
  sep = per-column spectral decomposition of image  -> (128, 128, 128)
  x = concat([sep, mask_embedding]) -> (134, 128, 128)
  conv0 4x4 s2 (134->256) + b0, lrelu          -> (256, 64, 64)
  conv1 4x4 s2 (256->512), inorm, lrelu        -> (512, 32, 32)
  conv2 4x4 s2 (512->1024), inorm, lrelu       -> (1024, 16, 16)
  conv3 4x4 s2 (1024->1024), inorm, lrelu      -> (1024, 8, 8)
  avgpool -> (1024,); head 1024->128 + b4      -> (128,)

Sharding: pure data parallel, batch 8 over 8 NeuronCores; weights replicated.

Math notes:
  sep[i,h,w] = colRT[i,h]*cos(2pi*i*w/W) + colJT[i,h]*sin(2pi*i*w/W)
    colRT = C @ img^T, colJT = S @ img^T,  C/S[w',i] = cos/sin(2pi*i*w'/W)/W
  (img^T is supplied pre-transposed from the host as "imgT".)
  b1..b3 cancel exactly through instance norm (constant channel shift), so
  they are ignored; b0 and b4 are applied.

fp8 strategy: all convs run as fp8e4m3 DoubleRow matmuls (2 fp8 MACs per PE
cell per cycle, K_eff=256).  Weights are pre-scaled by W_SCALE on the host;
activations between layers are stored as fp8 scaled by ACT_S.  Every conv
except conv0 is followed by InstanceNorm, which is scale-invariant, so these
scales cancel exactly (EPS is pre-scaled to match); conv0's scale is divided
out in its evacuation.  conv1..3 pair input-channel chunks (ci 2j,2j+1) per
DoubleRow matmul; conv0 pairs adjacent kw taps (same kh) since its sep input
is a single 128-channel chunk, and the 6 mask channels go through an
fp8-DoubleRow im2col matmul (rows paired 48x2) into the same PSUM group.

Evacuation uses a single Prelu activation per block: ACT_S*lrelu(s*(x-m)) ==
lrelu(ACT_S*s*x - ACT_S*s*m) via per-partition scale/bias APs, writing fp8
directly; conv3 additionally uses accum_out to produce the avgpool sums in
fp32 for free.  Prelu and Sqrt share every activation table set, so no
act-table reloads occur in steady state.
"""
from contextlib import ExitStack

import numpy as np
import ml_dtypes

import concourse.bass as bass
import concourse.tile as tile
from concourse import bacc, mybir

F32 = mybir.dt.float32
F16 = mybir.dt.float16
F8 = mybir.dt.float8e4
E4 = ml_dtypes.float8_e4m3
DR = mybir.MatmulPerfMode.DoubleRow
PRELU = mybir.ActivationFunctionType.Prelu

B, H, W = 8, 128, 128
EPS = 1e-5
ACT_S = 16.0  # fp8 activation scale between layers
W_SCALE = 1024.0  # fp8 weight scale (init is ~N(0, 0.02^2); 1024*|w| << 240)
S0C = ACT_S * W_SCALE  # conv PSUM scale (fp8 act x fp8 weight)
# inorm on S0C-scaled PSUM: (Sx - Sm)/sqrt(S^2 v + S^2 eps) == true inorm
EPS_SCALED = EPS * S0C * S0C

# conv output spatial sizes
S0, S1, S2, S3 = 64, 32, 16, 8


def _build_nc():
    nc = bacc.Bacc("TRN2", target_bir_lowering=False, debug=False)

    # ---------------- DRAM parameters (per-core) ----------------
    # packed fp16 spectral constants: [imgT, twC, twS, cos, sin] in one DMA
    d_spc = nc.dram_tensor("spc", [128, 5, 128], F16, kind="ExternalInput")
    # mask im2col fp8: [r(48), i(pair), oh, ow]
    d_maskim = nc.dram_tensor("maskim", [48, 2, S0, S0], F8, kind="ExternalInput")
    # conv0 sep weights: [p=ci, m, u(tap pair), i(pair member), co] fp8
    d_w0s = nc.dram_tensor("w0s", [128, 2, 8, 2, 128], F8, kind="ExternalInput")
    d_wm = nc.dram_tensor("wm", [48, 2, 256], F8, kind="ExternalInput")
    # convl weights: [m, j(ci-chunk pair), p=ci_lo, i(pair member), t, co] fp8
    d_w1 = nc.dram_tensor("w1l", [4, 1, 128, 2, 16, 128], F8, kind="ExternalInput")
    d_w2 = nc.dram_tensor("w2l", [8, 2, 128, 2, 16, 128], F8, kind="ExternalInput")
    d_w3 = nc.dram_tensor("w3l", [8, 4, 128, 2, 16, 128], F8, kind="ExternalInput")
    d_w4 = nc.dram_tensor("w4l", [8, 128, 128], F16, kind="ExternalInput")
    # packed biases: cols [ACT_S*b0(m=0), ACT_S*b0(m=1), b4]
    d_bt = nc.dram_tensor("bt", [128, 3], F32, kind="ExternalInput")
    d_out = nc.dram_tensor("out", [128], F32, kind="ExternalOutput")

    from contextlib import contextmanager

    @contextmanager
    def low_priority(tc, bump):
        orig = tc.cur_priority
        tc.cur_priority = orig + bump
        try:
            yield
        finally:
            tc.cur_priority = orig

    with tile.TileContext(nc) as tc, ExitStack() as ctx:
        const = ctx.enter_context(tc.tile_pool(name="const", bufs=1))
        act = ctx.enter_context(tc.tile_pool(name="act", bufs=1))
        wch = ctx.enter_context(tc.tile_pool(name="wch", bufs=7))
        ps = ctx.enter_context(tc.tile_pool(name="ps", bufs=3, space="PSUM"))
        psh = ctx.enter_context(tc.tile_pool(name="psh", bufs=1, space="PSUM"))
        tmp = ctx.enter_context(tc.tile_pool(name="tmp", bufs=6))
        tsp = ctx.enter_context(tc.tile_pool(name="tsp", bufs=3))

        # ------------- DMAs: critical-path first, then weight stream -------
        spc = const.tile([128, 5, 128], F16)
        nc.sync.dma_start(spc[:], d_spc.ap())
        imgT16 = spc[:, 0, :]
        twC = spc[:, 1, :]
        twS = spc[:, 2, :]
        c2ated = spc[:, 3, :]
        s2ated = spc[:, 4, :]
        w0t = const.tile([128, 2, 8, 2, 128], F8)
        nc.sync.dma_start(w0t[:], d_w0s.ap())
        wm_sb = const.tile([48, 2, 256], F8)
        nc.sync.dma_start(wm_sb[:], d_wm.ap())
        bt = const.tile([128, 3], F32)
        nc.sync.dma_start(bt[:], d_bt.ap())
        b0t = bt[:, 0:2]
        b4t = bt[:, 2:3]
        mask_im = act.tile([48, 2, S0, S0], F8)
        nc.sync.dma_start(mask_im[:], d_maskim.ap())
        w1t = const.tile([128, 4, 2, 16, 128], F8)
        nc.sync.dma_start(w1t[:], d_w1.ap().rearrange("m one p i t c -> p (m one) i t c"))
        # w2 fully resident, single big DMA
        w2t = const.tile([128, 16, 2, 16, 128], F8)
        nc.sync.dma_start(w2t[:], d_w2.ap().rearrange("m j p i t c -> p (m j) i t c"))
        # w3 streams in flat (m,j) order; first 14 pair-tiles through wch as
        # 7 two-tile slots (rest preloaded into recycled activation tiles
        # below, then wch slots as they free up)
        w3flat = d_w3.ap().rearrange("m j p i t c -> p (m j) i t c")
        wstream = {}
        for s in range(7):
            t_ = wch.tile([128, 2, 2, 16, 128], F8, tag="wch", name=f"w3s{s}")
            nc.sync.dma_start(t_[:], w3flat[:, 2 * s:2 * s + 2])
            for sub in range(2):
                wstream[divmod(2 * s + sub, 4)] = t_[:, sub]
        w4_sb = const.tile([128, 8, 128], F16)
        nc.sync.dma_start(w4_sb[:], d_w4.ap().rearrange("t k c -> k t c"))

        # ---------------- spectral map ----------------
        pR = ps.tile([128, 128], F32, tag="ps")
        nc.tensor.matmul(pR[:], twC, imgT16, start=True, stop=True)
        colRT = const.tile([128, 128], F16)
        nc.vector.tensor_copy(colRT[:], pR[:])
        pJ = ps.tile([128, 128], F32, tag="ps")
        nc.tensor.matmul(pJ[:], twS, imgT16, start=True, stop=True)
        colJT = const.tile([128, 128], F16)
        nc.vector.tensor_copy(colJT[:], pJ[:])

        # sep_pad holds ACT_S * sep (the ACT_S is folded into twC/twS on host)
        sep_pad = act.tile([128, H + 2, W + 2], F8)
        nc.gpsimd.memset(sep_pad[:, 0, :], 0.0)
        nc.gpsimd.memset(sep_pad[:, H + 1, :], 0.0)
        nc.gpsimd.memset(sep_pad[:, :, 0], 0.0)
        nc.gpsimd.memset(sep_pad[:, :, W + 1], 0.0)

        h_chunks = [4, 4, 8] + [16] * 7  # small first chunks: conv0 starts sooner
        h0 = 0
        for ci_, HC in enumerate(h_chunks):
            # A-term: colRT[i,h] bcast over w;  B-term: c2[i,w] bcast over h
            cR = colRT[:, h0:h0 + HC]
            aR = bass.AP(tensor=cR.tensor, offset=cR.offset,
                         ap=[cR.ap[0], [1, HC], [0, W]])
            cJ = colJT[:, h0:h0 + HC]
            aJ = bass.AP(tensor=cJ.tensor, offset=cJ.offset,
                         ap=[cJ.ap[0], [1, HC], [0, W]])
            b2 = bass.AP(tensor=c2ated.tensor, offset=c2ated.offset,
                         ap=[c2ated.ap[0], [0, HC], [1, W]])
            b3 = bass.AP(tensor=s2ated.tensor, offset=s2ated.offset,
                         ap=[s2ated.ap[0], [0, HC], [1, W]])
            t1 = tsp.tile([128, 16, W], F16, tag="tsp", name="t1")[:, :HC, :]
            nc.gpsimd.tensor_tensor(out=t1[:], in0=aR, in1=b2, op=mybir.AluOpType.mult)
            t2 = tsp.tile([128, 16, W], F16, tag="tsp", name="t2")[:, :HC, :]
            nc.vector.tensor_tensor(out=t2[:], in0=aJ, in1=b3, op=mybir.AluOpType.mult)
            eng = nc.vector if ci_ % 2 == 0 else nc.gpsimd
            eng.tensor_tensor(out=sep_pad[:, 1 + h0:1 + h0 + HC, 1:1 + W],
                              in0=t1[:], in1=t2[:], op=mybir.AluOpType.add)
            h0 += HC

        # ---------------- conv0: 134 -> 256, 128x128 -> 64x64 ----------------
        # sep part: fp8 DoubleRow over tap pairs (kw even/odd, same kh).
        # mask part: fp8 DoubleRow im2col matmul into the same PSUM group.
        c0pad = act.tile([128, 2, S0 + 2, S0 + 2], F8)
        with low_priority(tc, 400):
            for m in range(2):
                nc.gpsimd.memset(c0pad[:, m, 0, :], 0.0)
                nc.gpsimd.memset(c0pad[:, m, S0 + 1, :], 0.0)
                nc.gpsimd.memset(c0pad[:, m, :, 0], 0.0)
                nc.gpsimd.memset(c0pad[:, m, :, S0 + 1], 0.0)

        row = W + 2  # sep_pad row stride (elements)
        OHB0 = 8  # oh rows per chunk -> N = 8*64 = 512
        for ch in range(S0 // OHB0):
            oh0 = ch * OHB0
            for m in range(2):
                p0 = ps.tile([128, OHB0, S0], F32, tag="ps")
                for u in range(8):
                    kh, kw0 = (2 * u) // 4, (2 * u) % 4
                    base = sep_pad[:, kh + 2 * oh0, kw0]
                    rhs = bass.AP(tensor=base.tensor, offset=base.offset,
                                  ap=[base.ap[0], [1, 2], [2 * row, OHB0], [2, S0]])
                    nc.tensor.matmul(p0[:], w0t[:, m, u, :, :], rhs,
                                     start=(u == 0), stop=False, perf_mode=DR)
                nc.tensor.matmul(p0[:], wm_sb[:, :, m * 128:(m + 1) * 128],
                                 mask_im[:, :, oh0:oh0 + OHB0, :],
                                 start=False, stop=True, perf_mode=DR)
                # evac: ACT_S*lrelu(p/S0C + b0) = Prelu((ACT_S/S0C)p + ACT_S*b0)
                nc.scalar.activation(
                    out=c0pad[:, m, 1 + oh0:1 + oh0 + OHB0, 1:1 + S0],
                    in_=p0[:], func=PRELU, bias=b0t[:, m:m + 1],
                    scale=ACT_S / S0C, alpha=0.2)

        # ---------------- generic fp8 strided conv layer with inorm ----------
        # eps tiles: sqrt((v + EPS_SCALED)/ACT_S^2) -> reciprocal gives
        # ACT_S/sqrt(v+eps) directly (saves a multiply per block)
        eps_t = const.tile([128, 1], F32)
        nc.vector.memset(eps_t[:], EPS_SCALED)
        eps2_t = const.tile([128, 1], F32)
        nc.vector.memset(eps2_t[:], EPS_SCALED / (ACT_S * ACT_S))

        def conv_norm(x_pad, nk, nm, osz, w_at, out_pad=None, pooled=None,
                      post=None):
            """x_pad: tile (128, nk, isz+2, isz+2) fp8, chunks along dim1.
            out_pad: tile (128, nm, osz+2, osz+2) fp8 (scaled by ACT_S), or
            None with pooled (128, nm) fp32 spatial sums."""
            n_spatial = osz * osz
            ohb = max(1, min(osz, 512 // osz))
            nch = osz // ohb
            for m in range(nm):
                pm = ps.tile([128, osz, osz], F32, tag="ps")
                for ch in range(nch):
                    oh0 = ch * ohb
                    pslice = pm[:, oh0:oh0 + ohb, :]
                    first = True
                    for j in range(nk // 2):
                        wt = w_at(m, j)
                        for t in range(16):
                            kh, kw = t // 4, t % 4
                            rhs = x_pad[:, 2 * j:2 * j + 2,
                                        kh + 2 * oh0: kh + 2 * oh0 + 2 * ohb - 1: 2,
                                        kw: kw + 2 * osz - 1: 2]
                            nc.tensor.matmul(pslice, wt[:, :, t, :], rhs,
                                             start=first,
                                             stop=(j == nk // 2 - 1 and t == 15),
                                             perf_mode=DR)
                            first = False
                # instance norm stats over full spatial
                nsub = max(1, n_spatial // 512)
                sub = n_spatial // nsub
                stats = tmp.tile([128, nsub, 6], F32, tag="st")
                pf = pm[:].rearrange("p a b -> p (a b)")
                for s in range(nsub):
                    nc.vector.bn_stats(out=stats[:, s, :], in_=pf[:, s * sub:(s + 1) * sub])
                mv = tmp.tile([128, 2], F32, tag="mv")
                nc.vector.bn_aggr(out=mv[:], in_=stats[:])
                rsA = tmp.tile([128, 1], F32, tag="rsA")
                if out_pad is not None:
                    nc.scalar.activation(out=rsA[:], in_=mv[:, 1:2],
                                         func=mybir.ActivationFunctionType.Sqrt,
                                         bias=eps2_t[:],
                                         scale=1.0 / (ACT_S * ACT_S))
                else:
                    nc.scalar.activation(out=rsA[:], in_=mv[:, 1:2],
                                         func=mybir.ActivationFunctionType.Sqrt,
                                         bias=eps_t[:], scale=1.0)
                nc.vector.reciprocal(out=rsA[:], in_=rsA[:])
                nb = tmp.tile([128, 1], F32, tag="nb")
                nc.vector.tensor_scalar(out=nb[:], in0=mv[:, 0:1], scalar1=rsA[:],
                                        scalar2=-1.0, op0=mybir.AluOpType.mult,
                                        op1=mybir.AluOpType.mult)
                if out_pad is not None:
                    nc.scalar.activation(
                        out=out_pad[:, m, 1:1 + osz, 1:1 + osz],
                        in_=pm[:], func=PRELU, bias=nb[:], scale=rsA[:], alpha=0.2)
                else:
                    junk = tmp.tile([128, osz * osz], F16, tag="jk")
                    nc.scalar.activation(out=junk[:], in_=pf, func=PRELU,
                                         bias=nb[:], scale=rsA[:], alpha=0.2,
                                         accum_out=pooled[:, m:m + 1])
                if post is not None:
                    post(m)

        # conv1: 256 -> 512, 64x64 -> 32x32
        c1pad = act.tile([128, 4, S1 + 2, S1 + 2], F8)
        with low_priority(tc, 800):
            for m in range(4):
                nc.gpsimd.memset(c1pad[:, m, 0, :], 0.0)
                nc.gpsimd.memset(c1pad[:, m, S1 + 1, :], 0.0)
                nc.gpsimd.memset(c1pad[:, m, :, 0], 0.0)
                nc.gpsimd.memset(c1pad[:, m, :, S1 + 1], 0.0)
        conv_norm(c0pad, 2, 4, S1, lambda m, j: w1t[:, m, :, :, :], out_pad=c1pad)

        # w3 preload into recycled tiles, one merged DMA per landing area,
        # issued from the gpsimd queue (idle once sep construction is done)
        # so the tail of the weight stream runs on a second DMA ring in
        # parallel with the SP queue.  (sep_pad / tsp / mask_im die after
        # conv0; c0pad dies after conv1.)
        k = 14
        for pool_, tag_, n_ in [(act, "sep_pad", 4), (tsp, "tsp", 1),
                                (tsp, "tsp", 1), (tsp, "tsp", 1),
                                (act, "mask_im", 2), (act, "c0pad", 2)]:
            t_ = pool_.tile([128, n_, 2, 16, 128], F8, tag=tag_, name=f"w3r{k}")
            nc.gpsimd.dma_start(t_[:], w3flat[:, k:k + n_])
            for g in range(n_):
                wstream[divmod(k, 4)] = t_[:, g]
                k += 1

        # conv2: 512 -> 1024, 32x32 -> 16x16
        c2pad = act.tile([128, 8, S2 + 2, S2 + 2], F8)
        with low_priority(tc, 1200):
            for m in range(8):
                nc.gpsimd.memset(c2pad[:, m, 0, :], 0.0)
                nc.gpsimd.memset(c2pad[:, m, S2 + 1, :], 0.0)
                nc.gpsimd.memset(c2pad[:, m, :, 0], 0.0)
                nc.gpsimd.memset(c2pad[:, m, :, S2 + 1], 0.0)
        conv_norm(c1pad, 4, 8, S2, lambda m, j: w2t[:, 2 * m + j, :, :, :],
                  out_pad=c2pad)

        # rest of w3 rotates through wch pair-slots as conv3 consumes early
        # tiles, alternating between the sync and gpsimd DMA queues (both
        # are past their stream by now); programmed after the c2pad memsets
        # so the slot-recycle waits cannot head-of-line block gpsimd
        rot = 0
        while k < 32:
            n_ = min(2, 32 - k)
            t_ = wch.tile([128, 2, 2, 16, 128], F8, tag="wch", name=f"w3s{k}")
            eng_ = nc.sync if rot % 2 == 0 else nc.gpsimd
            eng_.dma_start(t_[:, 0:n_], w3flat[:, k:k + n_])
            rot += 1
            for g in range(n_):
                wstream[divmod(k, 4)] = t_[:, g]
                k += 1

        # conv3: 1024 -> 1024, 16x16 -> 8x8; only pooled sums survive.
        # The head matmul (out = w4^T @ pooled, w4 pre-scaled by 1/64) is
        # accumulated incrementally as each co-block's pooled sums appear.
        pooled32 = const.tile([128, 8], F32)
        pooled16 = const.tile([128, 8], F16)
        pH = psh.tile([128, 1], F32, tag="psH")

        def head_step(m):
            nc.vector.tensor_copy(pooled16[:, m:m + 1], pooled32[:, m:m + 1])
            nc.tensor.matmul(pH[:], w4_sb[:, m, :], pooled16[:, m:m + 1],
                             start=(m == 0), stop=(m == 7))

        conv_norm(c2pad, 8, 8, S3, lambda m, j: wstream[(m, j)],
                  pooled=pooled32, post=head_step)

        out_sb = const.tile([128, 1], F32)
        nc.vector.tensor_tensor(out=out_sb[:], in0=pH[:], in1=b4t[:],
                                op=mybir.AluOpType.add)
        nc.sync.dma_start(d_out.ap(), out_sb[:])

    nc.compile()
    return nc


_NC = None


def _get_nc():
    global _NC
    if _NC is None:
        _NC = _build_nc()
    return _NC


def _q8(x, scale):
    return np.clip(np.asarray(x, np.float32) * scale, -240.0, 240.0).astype(E4)


_SPC_CONST = None


def _spc_consts():
    """fp16 [4, 128, 128]: twC, twS (ACT_S folded), cos, sin tables."""
    global _SPC_CONST
    if _SPC_CONST is None:
        idx = np.arange(W)
        ang = (2.0 * np.pi / W) * np.outer(idx, idx).astype(np.float32)
        _SPC_CONST = np.stack([
            ACT_S * np.cos(ang) / W, ACT_S * np.sin(ang) / W,
            np.cos(ang), np.sin(ang)]).astype(np.float16)
    return _SPC_CONST


def _pack_spc(img):
    """img: (H, W) float -> packed fp16 (128, 5, 128): [imgT, twC, twS, cos, sin]."""
    spc = np.empty((128, 5, 128), np.float16)
    spc[:, 0, :] = np.asarray(img, np.float32).T.astype(np.float16)
    spc[:, 1:, :] = _spc_consts().transpose(1, 0, 2)
    return spc


def _prep_shared(w0, b0, w1, w2, w3, w4, b4):
    f16 = np.float16
    w0f = np.asarray(w0, np.float32)
    # w0 sep part: [p=ci, m, u, i, co] fp8, taps (2u, 2u+1) = (kh, kw0/kw0+1)
    w0s = np.empty((128, 2, 8, 2, 128), E4)
    for m in range(2):
        for u in range(8):
            kh, kw0 = (2 * u) // 4, (2 * u) % 4
            for i in range(2):
                w0s[:, m, u, i, :] = _q8(
                    w0f[128 * m:128 * (m + 1), 0:128, kh, kw0 + i].T, W_SCALE)
    # w0 mask part: rows r=(kh,kw,ci) paired 48x2, cols (m,co)
    wmf = np.zeros((96, 2, 128), np.float32)
    for kh in range(4):
        for kw in range(4):
            for m in range(2):
                wmf[(kh * 4 + kw) * 6:(kh * 4 + kw) * 6 + 6, m, :] = \
                    w0f[128 * m:128 * (m + 1), 128:134, kh, kw].T
    wm = _q8(wmf.reshape(48, 2, 256), W_SCALE)

    def pack8(wl, nm, nk):
        wlf = np.asarray(wl, np.float32)
        o = np.empty((nm, nk // 2, 128, 2, 16, 128), E4)
        for m in range(nm):
            for j in range(nk // 2):
                for i in range(2):
                    for t in range(16):
                        kh, kw = t // 4, t % 4
                        o[m, j, :, i, t, :] = _q8(
                            wlf[128 * m:128 * (m + 1),
                                128 * (2 * j + i):128 * (2 * j + i + 1),
                                kh, kw].T, W_SCALE)
        return o

    w1l = pack8(w1, 4, 2)
    w2l = pack8(w2, 8, 4)
    w3l = pack8(w3, 8, 8)
    w4f = np.asarray(w4, np.float32)[:, :, 0, 0] / (S3 * S3)  # (128, 1024)
    w4l = np.empty((8, 128, 128), f16)
    for kq in range(8):
        w4l[kq] = w4f[:, 128 * kq:128 * (kq + 1)].T.astype(f16)

    b0f = np.asarray(b0, np.float32)
    bt = np.stack([ACT_S * b0f[0:128], ACT_S * b0f[128:256],
                   np.asarray(b4, np.float32)], axis=1).astype(np.float32)
    return dict(w0s=w0s, wm=wm, w1l=w1l, w2l=w2l, w3l=w3l, w4l=w4l, bt=bt)


def _mask_imcol(mask_b):
    """mask_b: (6, H, W) float -> fp8 im2col (48, 2, 64, 64) scaled by ACT_S."""
    mp = np.zeros((6, H + 2, W + 2), np.float32)
    mp[:, 1:H + 1, 1:W + 1] = mask_b
    imcol = np.empty((96, S0, S0), np.float32)
    for kh in range(4):
        for kw in range(4):
            t = kh * 4 + kw
            imcol[t * 6:(t + 1) * 6] = mp[:, kh:kh + 2 * S0 - 1:2,
                                          kw:kw + 2 * S0 - 1:2]
    return _q8(imcol.reshape(48, 2, S0, S0), ACT_S)


def kernel(image, mask_embedding, w0, b0, w1, b1, w2, b2, w3, b3, w4, b4):
    from concourse.bass_utils import run_bass_kernel_spmd

    nc = _get_nc()
    shared = _prep_shared(w0, b0, w1, w2, w3, w4, b4)

    image = np.asarray(image, np.float32)
    mask = np.asarray(mask_embedding, np.float32)
    in_maps = []
    for b in range(B):
        m = dict(shared)
        m["spc"] = _pack_spc(image[b, 0])
        m["maskim"] = _mask_imcol(mask[b])
        in_maps.append(m)

    res = run_bass_kernel_spmd(nc, in_maps, list(range(B)))
    out = np.stack([res.results[b]["out"] for b in range(B)]).astype(np.float32)
    return out
